# revision 2
# baseline (speedup 1.0000x reference)
"""DDNLoss Trainium2 Bass kernel.

Computes fg_loss + bg_loss of the DDN depth-distribution focal loss:
    out = (1/(B*H*W)) * sum_pix  w_pix * ( -ALPHA * (1-p_t)^2 * log p_t )
with w_pix = 13 if pixel in any gt box else 1, p_t = softmax(depth_logits)[target].

Strategy: data-parallel over B across 8 NeuronCores, one batch image per core.
Per core the dominant cost is streaming the 81x192x640 f32 logits (40MB) once.

Key algebraic facts used:
  * target[pix] = min bin over matching instances, else NUM_BINS. Since bin(d)
    is monotonic in d, min-depth-then-bin == min-bin.  There are at most 17
    distinct target bins per image (16 instances + background bin 80).
  * p_t = E_t / S with E = exp(logits), S = sum_c E_c, E_t = E at target bin.
    log p_t = ln E_t - ln S, so no per-pixel gather is needed: E_t is a sum of
    the <=17 bin-planes of E masked by (target == bin), accumulated in PSUM by
    identity matmuls.
  * Channels are permuted HOST-side per core so the <=17 target-bin planes
    occupy static stream slots 0..16; per-image scalars (instance ids, bins)
    are runtime [96,1] operands, so a single NEFF serves all cores.

Pixel layout on device: fold the 192x640 map into [96, 1280] (p = pix//1280,
pure reshape), identical for logits planes, instance map and fg weights.
"""

import numpy as np

B, C, H, W = 8, 81, 192, 640
HI, WI = 384, 1280
NPI = 16
NUM_BINS = 80
DEPTH_MIN, DEPTH_MAX = 0.001, 60.0
ALPHA = 0.25
P = 96              # pixel-fold partitions
F = 1280            # pixel-fold free dim (H*W = P*F)
NSLOT = 17          # target-bin slots (16 instances + background)
CPT = 4             # channels (slots) per stream tile
DUMMY = 999.0       # mask scalar for unused bin slots (never matches)

_CACHE = {}


def _build_program():
    from contextlib import ExitStack

    import concourse.tile as tile
    from concourse import bacc, mybir

    nc = bacc.Bacc(
        "TRN2",
        target_bir_lowering=False,
        debug=False,
        enable_asserts=False,
        num_devices=8,
    )
    f32 = mybir.dt.float32
    i32 = mybir.dt.int32
    AF = mybir.ActivationFunctionType
    OP = mybir.AluOpType

    L = nc.dram_tensor("L", [P, C * F], f32, kind="ExternalInput").ap()
    inst = nc.dram_tensor("inst", [HI, WI], i32, kind="ExternalInput").ap()
    scal = nc.dram_tensor("scal", [P, 64], f32, kind="ExternalInput").ap()
    V2 = nc.dram_tensor("V2", [NPI, 2 * P], f32, kind="ExternalInput").ap()
    U = nc.dram_tensor("U", [NPI, W], f32, kind="ExternalInput").ap()
    ones = nc.dram_tensor("ones", [P, 1], f32, kind="ExternalInput").ap()
    I96 = nc.dram_tensor("I96", [P, P], f32, kind="ExternalInput").ap()
    out = nc.dram_tensor("out", [1, 1], f32, kind="ExternalOutput").ap()

    with tile.TileContext(nc) as tc, ExitStack() as ctx:
        cpool = ctx.enter_context(tc.tile_pool(name="consts", bufs=1))
        ppool = ctx.enter_context(tc.tile_pool(name="pix", bufs=1))
        lpool = ctx.enter_context(tc.tile_pool(name="stream", bufs=4))
        mpool = ctx.enter_context(tc.tile_pool(name="masked", bufs=2))
        pspool = ctx.enter_context(tc.tile_pool(name="ps", bufs=1, space="PSUM"))

        # ---- constants ----
        scal_t = cpool.tile([P, 64], f32)
        nc.sync.dma_start(scal_t[:], scal)
        V2_t = cpool.tile([NPI, 2 * P], f32)
        nc.sync.dma_start(V2_t[:], V2)
        U_t = cpool.tile([NPI, W], f32)
        nc.sync.dma_start(U_t[:], U)
        ones_t = cpool.tile([P, 1], f32)
        nc.sync.dma_start(ones_t[:], ones)
        I96_t = cpool.tile([P, P], f32)
        nc.sync.dma_start(I96_t[:], I96)

        # ---- instance map: rows 4p+2s+1 of [384,1280], cast int32->f32 ----
        # Full odd rows are DMA'd contiguously; the odd-column subsample is a
        # strided on-chip copy.
        inst_raw = ppool.tile([P, 2 * WI], f32, tag="instraw")
        inst_r = inst.rearrange("(p c) w -> p c w", c=4)
        for s in range(2):
            nc.gpsimd.dma_start(
                inst_raw[:, s * WI : (s + 1) * WI], inst_r[:, 2 * s + 1, :]
            )
        inst_f = ppool.tile([P, F], f32, tag="instf")
        inst_view = inst_raw[:].rearrange("p (f d) -> p f d", d=2)[:, :, 1]
        nc.vector.tensor_copy(inst_f[:], inst_view)

        # ---- fg box-count via rank-16 matmul: fg[p, s*640+w] ----
        fg_ps = [pspool.tile([P, W], f32, tag=f"ps{s}", name=f"fg{s}") for s in range(2)]
        for s in range(2):
            lhsT = V2_t[:, s * P : (s + 1) * P]
            nc.tensor.matmul(fg_ps[s][:, 0:512], lhsT, U_t[:, 0:512], start=True, stop=True)
            nc.tensor.matmul(fg_ps[s][:, 512:W], lhsT, U_t[:, 512:W], start=True, stop=True)
        wgt2 = ppool.tile([P, F], f32, tag="wgt2")  # 12 * (in any box)
        for s in range(2):
            nc.vector.tensor_scalar(
                wgt2[:, s * W : (s + 1) * W], fg_ps[s][:], 0.5, 12.0, OP.is_ge, OP.mult
            )

        # ---- target-map loop: acc = min_i (inst == id_i) * (bin_i - 80) ----
        acc = ppool.tile([P, F], f32, tag="acc")
        nc.vector.memset(acc[:], 0.0)
        cand = ppool.tile([P, F], f32, tag="cand")
        for i in range(NPI):
            nc.vector.tensor_scalar(
                cand[:], inst_f[:],
                scal_t[:, i : i + 1], scal_t[:, 16 + i : 17 + i],
                OP.is_equal, OP.mult,
            )
            nc.vector.tensor_tensor(acc[:], acc[:], cand[:], OP.min)

        # ---- main stream: exp, S/Et matmul accumulation ----
        S_ps = [pspool.tile([P, W], f32, tag=f"ps{s}", name=f"S{s}") for s in range(2)]
        Et_ps = [pspool.tile([P, W], f32, tag=f"et{s}", name=f"Et{s}") for s in range(2)]

        ntiles = (C + CPT - 1) // CPT
        slot = 0
        for j in range(ntiles):
            ns = min(CPT, C - j * CPT)
            lt = lpool.tile([P, CPT * F], f32, tag="lt", name=f"lt{j}")
            nc.sync.dma_start(
                lt[:, 0 : ns * F], L[:, j * CPT * F : (j * CPT + ns) * F]
            )
            nc.scalar.activation(lt[:, 0 : ns * F], lt[:, 0 : ns * F], AF.Exp)
            for k in range(ns):
                E = lt[:, k * F : (k + 1) * F]
                first, last = slot == 0, slot == C - 1
                for s in range(2):
                    nc.tensor.matmul(
                        S_ps[s][:, 0:512], I96_t[:], E[:, s * W : s * W + 512],
                        start=first, stop=last,
                    )
                    nc.tensor.matmul(
                        S_ps[s][:, 512:W], I96_t[:], E[:, s * W + 512 : (s + 1) * W],
                        start=first, stop=last,
                    )
                if slot < NSLOT:
                    mk = mpool.tile([P, F], f32, tag="mk", name=f"mk{slot}")
                    nc.vector.scalar_tensor_tensor(
                        mk[:], acc[:], scal_t[:, 32 + slot : 33 + slot], E,
                        OP.is_equal, OP.mult,
                    )
                    fb, lb = slot == 0, slot == NSLOT - 1
                    for s in range(2):
                        nc.tensor.matmul(
                            Et_ps[s][:, 0:512], I96_t[:], mk[:, s * W : s * W + 512],
                            start=fb, stop=lb,
                        )
                        nc.tensor.matmul(
                            Et_ps[s][:, 512:W], I96_t[:], mk[:, s * W + 512 : (s + 1) * W],
                            start=fb, stop=lb,
                        )
                slot += 1

        # ---- per-pixel focal loss ----
        lnS = ppool.tile([P, F], f32, tag="lnS")
        lnEt = ppool.tile([P, F], f32, tag="lnEt")
        for s in range(2):
            nc.scalar.activation(lnS[:, s * W : (s + 1) * W], S_ps[s][:], AF.Ln)
            nc.scalar.activation(lnEt[:, s * W : (s + 1) * W], Et_ps[s][:], AF.Ln)
        u = ppool.tile([P, F], f32, tag="u")  # log p_t  (<= 0)
        nc.vector.tensor_tensor(u[:], lnEt[:], lnS[:], OP.subtract)
        p = ppool.tile([P, F], f32, tag="p")
        nc.scalar.activation(p[:], u[:], AF.Exp)
        om2 = ppool.tile([P, F], f32, tag="om2")  # (1-p)^2
        nc.scalar.activation(om2[:], p[:], AF.Square, bias=1.0, scale=-1.0)
        core = ppool.tile([P, F], f32, tag="core")
        nc.vector.tensor_tensor(core[:], om2[:], u[:], OP.mult)
        # wl = (wgt2 + 1) * core ; rowsum = sum_f wl
        wl = ppool.tile([P, F], f32, tag="wl")
        rowsum = ppool.tile([P, 1], f32, tag="rows")
        nc.vector.scalar_tensor_tensor(
            wl[:], wgt2[:], 1.0, core[:], OP.add, OP.mult, accum_out=rowsum[:]
        )
        tot_ps = pspool.tile([1, 1], f32, tag="et0")
        nc.tensor.matmul(tot_ps[:], ones_t[:], rowsum[:], start=True, stop=True)
        res = ppool.tile([1, 1], f32, tag="res")
        nc.vector.tensor_copy(res[:], tot_ps[:])
        nc.sync.dma_start(out, res[:])

    nc.compile()
    return nc


def _get_program():
    if "nc" not in _CACHE:
        _CACHE["nc"] = _build_program()
    return _CACHE["nc"]


def _bins_f32(depths):
    """Per-instance target bin, replicating the reference's f32 arithmetic."""
    bin_size = 2.0 * (DEPTH_MAX - DEPTH_MIN) / (NUM_BINS * (1 + NUM_BINS))
    d = depths.astype(np.float32)
    q = (np.float32(8.0) * (d - np.float32(DEPTH_MIN))) / np.float32(bin_size)
    idx = np.float32(-0.5) + np.float32(0.5) * np.sqrt(np.float32(1.0) + q)
    bad = (idx < 0) | (idx > NUM_BINS) | ~np.isfinite(idx)
    return np.where(bad, np.float32(NUM_BINS), idx).astype(np.int32)


def make_in_maps(depth_logits, gt_boxes2d, gt_center_depth, instances, mask_instances):
    """Host-side shard prep: one input dict per core (= per batch image)."""
    logits = np.asarray(depth_logits, dtype=np.float32)
    boxes = np.asarray(gt_boxes2d, dtype=np.float32).reshape(B, NPI, 4)
    depths = np.asarray(gt_center_depth, dtype=np.float32).reshape(B, NPI)
    instances = np.asarray(instances, dtype=np.int32)
    mask_instances = np.asarray(mask_instances, dtype=np.int32)

    ones = np.ones((P, 1), np.float32)
    eye = np.eye(P, dtype=np.float32)
    ww = np.arange(W, dtype=np.int32)
    hh = np.arange(H, dtype=np.int32)

    in_maps = []
    for b in range(B):
        bins = _bins_f32(depths[b])                      # [16] int32
        ids_raw = (1000 + mask_instances[b]).astype(np.float32)
        dbin = (bins - NUM_BINS).astype(np.float32)      # bin - 80 in [-80, 0]

        ubins = np.unique(np.concatenate([bins, [NUM_BINS]])).astype(np.int64)
        rest = np.setdiff1d(np.arange(C, dtype=np.int64), ubins)
        perm = np.concatenate([ubins, rest])             # slots 0..16 = bin planes
        bprime = np.full(NSLOT, DUMMY, np.float32)
        bprime[: len(ubins)] = ubins.astype(np.float32) - NUM_BINS

        scal = np.zeros((P, 64), np.float32)
        scal[:, 0:NPI] = ids_raw[None, :]
        scal[:, 16 : 16 + NPI] = dbin[None, :]
        scal[:, 32 : 32 + NSLOT] = bprime[None, :]

        # boxes -> integer corners exactly as the reference (floor/ceil in f32)
        u1 = np.floor(boxes[b, :, 0]).astype(np.int32)
        v1 = np.floor(boxes[b, :, 1]).astype(np.int32)
        u2 = np.ceil(boxes[b, :, 2]).astype(np.int32)
        v2 = np.ceil(boxes[b, :, 3]).astype(np.int32)
        Um = ((ww[None, :] >= u1[:, None]) & (ww[None, :] < u2[:, None])).astype(np.float32)
        v_ind = ((hh[None, :] >= v1[:, None]) & (hh[None, :] < v2[:, None])).astype(np.float32)
        # V2[i, s*96 + p] = v_ind[i, 2p+s]
        V2m = np.ascontiguousarray(
            v_ind.reshape(NPI, P, 2).transpose(0, 2, 1).reshape(NPI, 2 * P)
        )

        # logits: permute channels, fold each plane [192,640] -> [96,1280]
        Ls = np.ascontiguousarray(
            logits[b][perm].reshape(C, P, F).transpose(1, 0, 2).reshape(P, C * F)
        )

        in_maps.append(
            {
                "L": Ls,
                "inst": np.ascontiguousarray(instances[b]),
                "scal": scal,
                "V2": V2m,
                "U": np.ascontiguousarray(Um),
                "ones": ones,
                "I96": eye,
            }
        )
    return in_maps


def kernel(depth_logits, gt_boxes2d, gt_center_depth, instances, mask_instances):
    from concourse.bass_utils import run_bass_kernel_spmd

    nc = _get_program()
    in_maps = make_in_maps(
        depth_logits, gt_boxes2d, gt_center_depth, instances, mask_instances
    )
    r = run_bass_kernel_spmd(
        nc, in_maps, core_ids=list(range(8)), trace=_CACHE.get("trace", False)
    )
    _CACHE["last_results"] = r
    total = float(sum(res["out"][0, 0].astype(np.float64) for res in r.results))
    val = -ALPHA * total / float(B * H * W)
    return np.float32(val)
